# revision 6
# baseline (speedup 1.0000x reference)
"""DIEN kernel for Trainium2 (Bass/Tile), 8-way data-parallel over batch.

Layout: transposed activations [feature (<=128 partitions), batch (free dim)].
Per core: 512 batch rows, T=50 steps. GRU / attention / AUGRU fused in one
skewed loop (ATT runs 1 step behind GRU, AUGRU 6 behind), head at the end.

Matmuls run in float32r (full-rate single-pass fp32, ~1.6e-4 rel rounding);
weights are the natural [in, out] layout = the stationary lhsT operand, so
the whole kernel needs no transposes (host pre-transposes the input once).
"""
import sys

sys.path.insert(0, "/opt/trn_rl_repo")

import numpy as np

import concourse.bass as bass
import concourse.mybir as mybir
import concourse.tile as tile
from concourse import bacc
from concourse.bass_utils import run_bass_kernel_spmd

B, T, D, U = 4096, 50, 128, 128
NCORES = 8
BL = B // NCORES  # 512
P = 128
F32 = mybir.dt.float32
F32R = mybir.dt.float32r
BF16 = mybir.dt.bfloat16
AF = mybir.ActivationFunctionType
OP = mybir.AluOpType
LEAKY = 0.0003
SKEW_ATT = 1   # ATT processes step i-1
SKEW_AU = 6    # AUGRU processes step i-6
NITER = T + SKEW_AU  # 56

# ---------------------------------------------------------------- config
CFG = {
    "gate_dt": F32,      # z, r, u, r2, zc, us, u_, ats
    "cand_dt": F32,      # s-pair, hc/c pair, t1/t2, q, p
    # engine per op: "v" = vector(DVE), "g" = gpsimd(Pool)
    "eng": {
        "zc": "v", "p": "g", "q": "v", "hn": "v",
        "us": "g", "p2": "g", "q2": "v", "hn2": "v",
        "u_": "g", "mmj": "g",
        "t1": "v", "s1": "v", "t2": "v", "s2": "v",
        "relu2": "v",
    },
    "relu1_act": True,   # relu1 on ACT (else DVE tensor_scalar)
    "tanh_pair": True,
    "debug": False,
}

# bias column indices in the packed [128, 16] bias tensor
BZ, BR, BU, BR2, B1H, B0H, BC, B1, B2, B3, DB1A, DB1B, DB2, FB = range(14)


def _eng(nc, key):
    return nc.vector if CFG["eng"][key] == "v" else nc.gpsimd


def build_nc(debug=False):
    nc = bacc.Bacc()
    GD = CFG["gate_dt"]
    CD = CFG["cand_dt"]

    # ---------------- DRAM params (f32r so DMA-fed tiles can feed matmuls)
    hist = nc.dram_tensor("hist", [T, P, BL], F32R, kind="ExternalInput")
    news = nc.dram_tensor("news", [P, BL], F32R, kind="ExternalInput")
    w_gw = nc.dram_tensor("w_gw", [P, 3 * U], F32R, kind="ExternalInput")
    w_gu = nc.dram_tensor("w_gu", [P, 3 * U], F32R, kind="ExternalInput")
    w_aw = nc.dram_tensor("w_aw", [P, 3 * U], F32R, kind="ExternalInput")
    w_au = nc.dram_tensor("w_au", [P, 3 * U], F32R, kind="ExternalInput")
    w_ax = nc.dram_tensor("w_ax", [P, P], F32R, kind="ExternalInput")
    w_am = nc.dram_tensor("w_am", [P, P], F32R, kind="ExternalInput")
    w_abc = nc.dram_tensor("w_abc", [P, P], F32R, kind="ExternalInput")
    w_a2 = nc.dram_tensor("w_a2", [P, 64], F32R, kind="ExternalInput")
    # 4 one-hot-column copies of att_W3: block k is [64,4] with column k = W3.
    # Accumulating 4 steps into one [4,BL] psum puts step 4g+k's logit on
    # partition k, so one sigmoid covers 4 steps.
    w_a3 = nc.dram_tensor("w_a3", [64, 16], F32R, kind="ExternalInput")
    w_d1a = nc.dram_tensor("w_d1a", [P, 256], F32R, kind="ExternalInput")
    w_d1b = nc.dram_tensor("w_d1b", [P, 256], F32R, kind="ExternalInput")
    w_d2a = nc.dram_tensor("w_d2a", [P, P], F32R, kind="ExternalInput")
    w_d2b = nc.dram_tensor("w_d2b", [P, P], F32R, kind="ExternalInput")
    w_f = nc.dram_tensor("w_f", [P, 1], F32R, kind="ExternalInput")
    rowsel = nc.dram_tensor("rowsel", [4, 512], F32R, kind="ExternalInput")
    biases = nc.dram_tensor("biases", [P, 16], F32, kind="ExternalInput")
    y_out = nc.dram_tensor("y", [1, BL], F32, kind="ExternalOutput")
    if debug:
        hg_out = nc.dram_tensor("hg", [P, BL], F32R, kind="ExternalOutput")
        h2_out = nc.dram_tensor("h2f", [P, BL], F32R, kind="ExternalOutput")

    with tile.TileContext(nc) as tc:
        import contextlib

        ctx = contextlib.ExitStack()
        with ctx:
            wp = ctx.enter_context(tc.tile_pool(name="wp", bufs=1))
            xp = ctx.enter_context(tc.tile_pool(name="xp", bufs=4))
            hsp = ctx.enter_context(tc.tile_pool(name="hsp", bufs=8))
            gp = ctx.enter_context(tc.tile_pool(name="gp", bufs=2))
            ps = ctx.enter_context(tc.tile_pool(name="ps", bufs=1, space="PSUM"))
            ps1b = ctx.enter_context(tc.tile_pool(name="ps1b", bufs=1, space="PSUM"))
            ps2 = ps1b

            # ---------------- load weights/biases
            def wtile(name, dram, shape, dt=F32R):
                t = wp.tile(shape, dt, name=name, tag=name)
                nc.sync.dma_start(t[:], dram[:])
                return t

            gw = wtile("gw", w_gw, [P, 3 * U])
            gu = wtile("gu", w_gu, [P, 3 * U])
            aw = wtile("aw", w_aw, [P, 3 * U])
            au = wtile("au", w_au, [P, 3 * U])
            ax = wtile("ax", w_ax, [P, P])
            am = wtile("am", w_am, [P, P])
            abc = wtile("abc", w_abc, [P, P])
            a2w = wtile("a2w", w_a2, [P, 64])
            a3w = wtile("a3w", w_a3, [64, 16])
            d1a = wtile("d1a", w_d1a, [P, 256])
            d1b = wtile("d1b", w_d1b, [P, 256])
            d2a = wtile("d2a", w_d2a, [P, P])
            d2b = wtile("d2b", w_d2b, [P, P])
            fw = wtile("fw", w_f, [P, 1])
            rsel = wtile("rsel", rowsel, [4, 512])
            bia = wtile("bia", biases, [P, 16], F32)
            news_r = wtile("news_r", news, [P, BL])

            def bap(col, rows=P):
                return bia[0:rows, col : col + 1]

            # ---------------- state
            hs_tiles = {}   # t -> tile [P, BL] f32r (GRU outputs, ring of 8)
            h2_tiles = {}   # s -> tile (AUGRU state, ring of 2)
            ats_tiles = {}  # group g -> [4, BL] tile
            a3_psums = {}   # group g -> [4, BL] psum

            zero_h = hsp.tile([P, BL], F32R, name="h_init", tag="hs")
            nc.vector.memset(zero_h[:].bitcast(F32), 0.0)
            zero_h2 = gp.tile([P, BL], F32R, name="h2_init", tag="h2")
            nc.vector.memset(zero_h2[:].bitcast(F32), 0.0)
            hs_tiles[-1] = zero_h
            h2_tiles[-1] = zero_h2

            mm = nc.tensor.matmul

            for i in range(NITER):
                t = i if i < T else None
                j = i - SKEW_ATT if 0 <= i - SKEW_ATT < T else None
                s = i - SKEW_AU if 0 <= i - SKEW_AU < T else None

                # ---------------- GRU step t
                if t is not None:
                    h_prev = hs_tiles[t - 1]
                    x_t = xp.tile([P, BL], F32R, name=f"x{t}", tag="x")
                    nc.sync.dma_start(x_t[:], hist[t])

                    pz = ps.tile([P, BL], F32, name=f"pz{t}", tag="g1")
                    mm(pz[:], gw[:, 0:U], x_t[:], start=True, stop=False)
                    mm(pz[:], gu[:, 0:U], h_prev[:], start=False, stop=True)
                    pr = ps.tile([P, BL], F32, name=f"pr{t}", tag="g2")
                    mm(pr[:], gw[:, U : 2 * U], x_t[:], start=True, stop=False)
                    mm(pr[:], gu[:, U : 2 * U], h_prev[:], start=False, stop=True)
                    pxh = ps.tile([P, BL], F32, name=f"pxh{t}", tag="g3")
                    mm(pxh[:], gw[:, 2 * U : 3 * U], x_t[:], start=True, stop=True)
                    phh = ps.tile([P, BL], F32, name=f"phh{t}", tag="g4")
                    mm(phh[:], gu[:, 2 * U : 3 * U], h_prev[:], start=True, stop=True)

                    z = gp.tile([P, BL], GD, name=f"z{t}", tag="z")
                    nc.scalar.activation(z[:], pz[:], AF.Sigmoid, bias=bap(BZ))
                    r = gp.tile([P, BL], GD, name=f"r{t}", tag="r")
                    nc.scalar.activation(r[:], pr[:], AF.Sigmoid, bias=bap(BR))

                    t1 = gp.tile([P, BL], CD, name=f"t1{t}", tag="t1")
                    _eng(nc, "t1").scalar_tensor_tensor(
                        t1[:], phh[:], bap(B1H), r[:], OP.add, OP.mult
                    )
                    # s-pair left: s1 = (pxh + b0h) + t1
                    sp = gp.tile([P, 2 * BL], CD, name=f"sp{i}", tag="sp")
                    _eng(nc, "s1").scalar_tensor_tensor(
                        sp[:, 0:BL], pxh[:], bap(B0H), t1[:], OP.add, OP.add
                    )
                    zc = gp.tile([P, BL], GD, name=f"zc{t}", tag="zc")
                    _eng(nc, "zc").tensor_scalar(
                        zc[:], z[:], -1.0, 1.0, OP.mult, OP.add
                    )
                    p = gp.tile([P, BL], CD, name=f"p{t}", tag="p")
                    _eng(nc, "p").tensor_mul(p[:], z[:], h_prev[:])

                # ---------------- AUGRU recurrent-side matmuls + gates (step s)
                if s is not None:
                    h2_prev = h2_tiles[s - 1]
                    hs_s = hs_tiles[s]
                    pu = ps.tile([P, BL], F32, name=f"pu{s}", tag="g1")
                    mm(pu[:], aw[:, 0:U], hs_s[:], start=True, stop=False)
                    mm(pu[:], au[:, 0:U], h2_prev[:], start=False, stop=True)
                    pr2 = ps.tile([P, BL], F32, name=f"pr2{s}", tag="g2")
                    mm(pr2[:], aw[:, U : 2 * U], hs_s[:], start=True, stop=False)
                    mm(pr2[:], au[:, U : 2 * U], h2_prev[:], start=False, stop=True)
                    pxc = ps.tile([P, BL], F32, name=f"pxc{s}", tag="g3")
                    mm(pxc[:], aw[:, 2 * U : 3 * U], hs_s[:], start=True, stop=True)
                    prc = ps.tile([P, BL], F32, name=f"prc{s}", tag="g4")
                    mm(prc[:], au[:, 2 * U : 3 * U], h2_prev[:], start=True, stop=True)

                    u = gp.tile([P, BL], GD, name=f"u{s}", tag="u")
                    nc.scalar.activation(u[:], pu[:], AF.Sigmoid, bias=bap(BU))
                    r2 = gp.tile([P, BL], GD, name=f"r2{s}", tag="r2")
                    nc.scalar.activation(r2[:], pr2[:], AF.Sigmoid, bias=bap(BR2))

                    t2 = gp.tile([P, BL], CD, name=f"t2{s}", tag="t2")
                    _eng(nc, "t2").tensor_mul(t2[:], prc[:], r2[:])
                    if t is not None:
                        sp2_dst = sp[:, BL : 2 * BL]
                    else:
                        sp = gp.tile([P, 2 * BL], CD, name=f"sp{i}", tag="sp")
                        sp2_dst = sp[:, BL : 2 * BL]
                    _eng(nc, "s2").scalar_tensor_tensor(
                        sp2_dst, pxc[:], bap(BC), t2[:], OP.add, OP.add
                    )
                    g = s // 4
                    pat = ps1b.tile([P, BL], F32, name=f"pat{s}", tag="atrep")
                    mm(pat[:], rsel[:, P * (s % 4) : P * (s % 4) + P],
                       ats_tiles[g][:], start=True, stop=True)
                    u_ = gp.tile([P, BL], GD, name=f"u_{s}", tag="u_")
                    nc.vector.tensor_mul(u_[:], u[:], pat[:])
                    us = gp.tile([P, BL], GD, name=f"us{s}", tag="us")
                    _eng(nc, "us").tensor_scalar(
                        us[:], u_[:], -1.0, 1.0, OP.mult, OP.add
                    )
                    p2 = gp.tile([P, BL], CD, name=f"p2{s}", tag="p2")
                    _eng(nc, "p2").tensor_mul(p2[:], us[:], h2_prev[:])

                # ---------------- tanh (paired when both scans active)
                if t is not None or s is not None:
                    hcp = gp.tile([P, 2 * BL], CD, name=f"hcp{i}", tag="hcp")
                    if t is not None and s is not None:
                        nc.scalar.activation(hcp[:], sp[:], AF.Tanh)
                    elif t is not None:
                        nc.scalar.activation(hcp[:, 0:BL], sp[:, 0:BL], AF.Tanh)
                    else:
                        nc.scalar.activation(
                            hcp[:, BL : 2 * BL], sp[:, BL : 2 * BL], AF.Tanh
                        )

                # ---------------- GRU blend -> hs[t]
                if t is not None:
                    hc = hcp[:, 0:BL]
                    q = gp.tile([P, BL], CD, name=f"q{t}", tag="q")
                    _eng(nc, "q").tensor_mul(q[:], zc[:], hc)
                    hn = hsp.tile([P, BL], F32R, name=f"h{t}", tag="hs")
                    _eng(nc, "hn").tensor_add(hn[:], q[:], p[:])
                    hs_tiles[t] = hn
                    if t >= 8:
                        del hs_tiles[t - 8]

                # ---------------- AUGRU blend -> h2[s]
                if s is not None:
                    c = hcp[:, BL : 2 * BL]
                    q2 = gp.tile([P, BL], CD, name=f"q2{s}", tag="q2")
                    _eng(nc, "q2").tensor_mul(q2[:], u_[:], c)
                    hn2 = gp.tile([P, BL], F32R, name=f"h2_{s}", tag="h2")
                    _eng(nc, "hn2").tensor_add(hn2[:], q2[:], p2[:])
                    h2_tiles[s] = hn2
                    if s - 2 in h2_tiles:
                        del h2_tiles[s - 2]

                # ---------------- attention step j
                if j is not None:
                    hs_j = hs_tiles[j]
                    mmj = gp.tile([P, BL], F32R, name=f"mmj{j}", tag="mmj")
                    _eng(nc, "mmj").tensor_mul(mmj[:], hs_j[:], news_r[:])
                    pa1 = ps1b.tile([P, BL], F32, name=f"pa1{j}", tag="a1")
                    mm(pa1[:], ax[:], hs_j[:], start=True, stop=False)
                    mm(pa1[:], am[:], mmj[:], start=False, stop=False)
                    mm(pa1[:], abc[:], news_r[:], start=False, stop=True)
                    a1 = gp.tile([P, BL], F32R, name=f"a1{j}", tag="a1s")
                    if CFG["relu1_act"]:
                        nc.scalar.activation(a1[:], pa1[:], AF.Relu, bias=bap(B1))
                    else:
                        nc.vector.tensor_scalar(
                            a1[:], pa1[:], bap(B1), 0.0, OP.add, OP.max
                        )
                    pa2 = ps1b.tile([64, BL], F32, name=f"pa2{j}", tag="a2")
                    mm(pa2[:], a2w[:], a1[:], start=True, stop=True)
                    a2 = gp.tile([64, BL], F32R, name=f"a2{j}", tag="a2s")
                    _eng(nc, "relu2").tensor_scalar(
                        a2[:], pa2[:], bap(B2, rows=64), 0.0, OP.add, OP.max
                    )
                    g = j // 4
                    k4 = j % 4
                    if k4 == 0:
                        a3_psums[g] = ps1b.tile([4, BL], F32, name=f"pa3{g}", tag="a3")
                    mm(
                        a3_psums[g][:],
                        a3w[:, 4 * k4 : 4 * k4 + 4],
                        a2[:],
                        start=(k4 == 0),
                        stop=(k4 == 3 or j == T - 1),
                    )
                    if j % 4 == 3 or j == T - 1:
                        k = (j % 4) + 1
                        ats = gp.tile([4, BL], F32R, name=f"ats{g}", tag="ats")
                        nc.scalar.activation(
                            ats[0:k, :],
                            a3_psums[g][0:k, :],
                            AF.Sigmoid,
                            bias=bap(B3, rows=k),
                        )
                        ats_tiles[g] = ats

            # ---------------- deep head
            h2f = h2_tiles[T - 1]
            if debug:
                nc.sync.dma_start(hg_out[:], hs_tiles[T - 1][:])
                nc.sync.dma_start(h2_out[:], h2f[:])

            o1 = gp.tile([P, 2 * BL], F32R, name="o1", tag="o1")
            for mch in range(2):
                po = ps.tile([P, BL], F32, name=f"po1_{mch}", tag="g1")
                mm(po[:], d1a[:, mch * P : (mch + 1) * P], h2f[:], start=True, stop=False)
                mm(po[:], d1b[:, mch * P : (mch + 1) * P], news_r[:], start=False, stop=True)
                nc.scalar.activation(
                    o1[:, mch * BL : (mch + 1) * BL],
                    po[:],
                    AF.Lrelu,
                    bias=bap(DB1A + mch),
                    alpha=LEAKY,
                )
            po2 = ps.tile([P, BL], F32, name="po2", tag="g2")
            mm(po2[:], d2a[:], o1[:, 0:BL], start=True, stop=False)
            mm(po2[:], d2b[:], o1[:, BL : 2 * BL], start=False, stop=True)
            o2 = gp.tile([P, BL], F32R, name="o2", tag="o2")
            nc.scalar.activation(o2[:], po2[:], AF.Lrelu, bias=bap(DB2), alpha=LEAKY)
            py = ps.tile([1, BL], F32, name="py", tag="g3")
            mm(py[:], fw[:], o2[:], start=True, stop=True)
            y_sb = gp.tile([1, BL], F32, name="y_sb", tag="ysb")
            nc.scalar.activation(y_sb[:], py[:], AF.Sigmoid, bias=bap(FB, rows=1))
            nc.sync.dma_start(y_out[:], y_sb[:])

    nc.compile()
    return nc


def _a3_onehot(att_W3):
    w = np.zeros((64, 16), np.float32)
    for k in range(4):
        w[:, 4 * k + k] = att_W3[:, 0]
    return w


def prep_inputs(inputs_np, gru_W, gru_U, gru_b, att_W1, att_b1, att_W2, att_b2,
                att_W3, att_b3, au_Wu, au_bu, au_Uu, au_Wr, au_br, au_Ur,
                au_Wc, au_bc, au_Uc, bn_gamma, bn_beta, bn_mean, bn_var,
                d_W1, d_b1, d_W2, d_b2, f_W, f_b):
    """Host-side preprocessing. Returns (shared weight map, per-core input maps)."""
    f32 = np.float32

    biases = np.zeros((P, 16), f32)
    biases[:, BZ] = gru_b[0, 0:U] + gru_b[1, 0:U]
    biases[:, BR] = gru_b[0, U : 2 * U] + gru_b[1, U : 2 * U]
    biases[:, BU] = au_bu
    biases[:, BR2] = au_br
    biases[:, B1H] = gru_b[1, 2 * U : 3 * U]
    biases[:, B0H] = gru_b[0, 2 * U : 3 * U]
    biases[:, BC] = au_bc
    biases[:, B1] = att_b1
    biases[0:64, B2] = att_b2
    biases[0:4, B3] = att_b3[0]

    # BN folded into layer 1
    s = (bn_gamma / np.sqrt(bn_var + 1e-3)).astype(f32)
    tt = (bn_beta - bn_mean * s).astype(f32)
    W1p = (s[:, None] * d_W1).astype(f32)
    b1p = (tt @ d_W1 + d_b1).astype(f32)
    biases[:, DB1A] = b1p[0:P]
    biases[:, DB1B] = b1p[P : 2 * P]
    biases[:, DB2] = d_b2
    biases[0, FB] = f_b[0]

    shared = {
        "w_gw": np.ascontiguousarray(gru_W, f32),
        "w_gu": np.ascontiguousarray(gru_U, f32),
        "w_aw": np.ascontiguousarray(np.concatenate([au_Wu, au_Wr, au_Wc], axis=1), f32),
        "w_au": np.ascontiguousarray(np.concatenate([au_Uu, au_Ur, au_Uc], axis=1), f32),
        "w_ax": np.ascontiguousarray(att_W1[0:P] - att_W1[P : 2 * P], f32),
        "w_am": np.ascontiguousarray(att_W1[3 * P : 4 * P], f32),
        "w_abc": np.ascontiguousarray(att_W1[P : 2 * P] + att_W1[2 * P : 3 * P], f32),
        "w_a2": np.ascontiguousarray(att_W2, f32),
        "w_a3": _a3_onehot(att_W3),
        "w_d1a": np.ascontiguousarray(W1p[0:P], f32),
        "w_d1b": np.ascontiguousarray(W1p[P : 2 * P], f32),
        "w_d2a": np.ascontiguousarray(d_W2[0:P], f32),
        "w_d2b": np.ascontiguousarray(d_W2[P : 2 * P], f32),
        "w_f": np.ascontiguousarray(f_W, f32),
        "biases": biases,
    }

    rowsel = np.zeros((4, 512), f32)
    for k in range(4):
        rowsel[k, P * k : P * k + P] = 1.0
    shared["rowsel"] = rowsel

    in_maps = []
    for c in range(NCORES):
        sh = inputs_np[c * BL : (c + 1) * BL]  # [BL, T+1, D]
        hist_t = np.ascontiguousarray(sh[:, :T, :].transpose(1, 2, 0), f32)  # [T,D,BL]
        news_t = np.ascontiguousarray(sh[:, T, :].T, f32)  # [D, BL]
        m = dict(shared)
        m["hist"] = hist_t
        m["news"] = news_t
        in_maps.append(m)
    return in_maps


_NC_CACHE = {}


def get_nc(debug=False):
    key = (debug,)
    if key not in _NC_CACHE:
        _NC_CACHE[key] = build_nc(debug=debug)
    return _NC_CACHE[key]


def kernel(**inputs):
    inputs = {k: np.asarray(v) for k, v in inputs.items()}
    in_maps = prep_inputs(
        inputs["inputs"], inputs["gru_W"], inputs["gru_U"], inputs["gru_b"],
        inputs["att_W1"], inputs["att_b1"], inputs["att_W2"], inputs["att_b2"],
        inputs["att_W3"], inputs["att_b3"], inputs["au_Wu"], inputs["au_bu"],
        inputs["au_Uu"], inputs["au_Wr"], inputs["au_br"], inputs["au_Ur"],
        inputs["au_Wc"], inputs["au_bc"], inputs["au_Uc"], inputs["bn_gamma"],
        inputs["bn_beta"], inputs["bn_mean"], inputs["bn_var"], inputs["d_W1"],
        inputs["d_b1"], inputs["d_W2"], inputs["d_b2"], inputs["f_W"],
        inputs["f_b"],
    )
    nc = get_nc(debug=CFG["debug"])
    res = run_bass_kernel_spmd(nc, in_maps, list(range(NCORES)))
    y = np.concatenate(
        [res.results[c]["y"].reshape(-1)[:, None] for c in range(NCORES)], axis=0
    ).astype(np.float32)
    return y


# revision 12
# speedup vs baseline: 1.2654x; 1.2654x over previous
"""DIEN kernel for Trainium2 (Bass/Tile), 8-way data-parallel over batch.

Layout: transposed activations [feature (<=128 partitions), batch (free dim)].
Per core: 512 batch rows, T=50 steps. GRU / attention / AUGRU fused in one
skewed loop (ATT runs 1 step behind GRU, AUGRU 6 behind), head at the end.

Matmuls run in float32r (full-rate single-pass fp32, ~1.6e-4 rel rounding);
weights are the natural [in, out] layout = the stationary lhsT operand, so
the whole kernel needs no transposes (host pre-transposes the input once).

Per-step structure (steady state, all tiles [128, 512] unless noted):
  sigmoid gates come from a merged 2-bank PSUM pair (z|r and u|r2) with
  their biases pre-added by K=1 rank-1 matmuls, so one ACT op covers two
  gates. The tanh candidate is accumulated fully in PSUM: the r*(Uh h+b1h)
  term is written back into the Wh x PSUM with an identity matmul, so tanh
  reads PSUM directly (x-side bias via the ACT bias port).
"""
import sys

sys.path.insert(0, "/opt/trn_rl_repo")

import numpy as np

import concourse.bass as bass
import concourse.mybir as mybir
import concourse.tile as tile
from concourse import bacc
from concourse.bass_utils import run_bass_kernel_spmd

B, T, D, U = 4096, 50, 128, 128
NCORES = 8
BL = B // NCORES  # 512
P = 128
F32 = mybir.dt.float32
F32R = mybir.dt.float32r
BF16 = mybir.dt.bfloat16
AF = mybir.ActivationFunctionType
OP = mybir.AluOpType
LEAKY = 0.0003
SKEW_ATT = 1   # ATT processes step i-1
SKEW_AU = 6    # AUGRU processes step i-6
NITER = T + SKEW_AU  # 56

# ---------------------------------------------------------------- config
CFG = {
    "gate_dt": BF16,     # z, r, u, r2, zc, us, u_
    "cand_dt": BF16,     # hc, c, q, p
    # engine per op: "v" = vector(DVE), "g" = gpsimd(Pool)
    "eng": {
        "zc": "v", "p": "g", "q": "v", "hn": "v",
        "us": "v", "p2": "v", "q2": "v", "hn2": "v",
        "u_": "v", "mmj": "g",
        "t1": "v", "t2": "v",
        "relu2": "v",
    },
    "relu1_act": True,   # relu1 on ACT (else DVE tensor_scalar)
    "merge_zr": True,    # merged sigmoid for z|r (bias via K=1 matmuls)
    "merge_ur": True,    # merged sigmoid for u|r2
    "debug": False,
}

# bias column indices in the packed [128, 16] bias tensor
BZ, BR, BU, BR2, B1H, B0H, BC, B1, B2, B3, DB1A, DB1B, DB2, FB = range(14)


def _eng(nc, key):
    return nc.vector if CFG["eng"][key] == "v" else nc.gpsimd


def build_nc(debug=False):
    nc = bacc.Bacc()
    GD = CFG["gate_dt"]
    CD = CFG["cand_dt"]

    # ---------------- DRAM params (f32r so DMA-fed tiles can feed matmuls)
    hist = nc.dram_tensor("hist", [T, P, BL], F32R, kind="ExternalInput")
    news = nc.dram_tensor("news", [P, BL], F32R, kind="ExternalInput")
    w_gw = nc.dram_tensor("w_gw", [P, 3 * U], F32R, kind="ExternalInput")
    w_gu = nc.dram_tensor("w_gu", [P, 3 * U], F32R, kind="ExternalInput")
    w_aw = nc.dram_tensor("w_aw", [P, 3 * U], F32R, kind="ExternalInput")
    w_au = nc.dram_tensor("w_au", [P, 3 * U], F32R, kind="ExternalInput")
    w_ax = nc.dram_tensor("w_ax", [P, P], F32R, kind="ExternalInput")
    w_am = nc.dram_tensor("w_am", [P, P], F32R, kind="ExternalInput")
    w_abc = nc.dram_tensor("w_abc", [P, P], F32R, kind="ExternalInput")
    w_a2 = nc.dram_tensor("w_a2", [P, 64], F32R, kind="ExternalInput")
    # 4 one-hot-column copies of att_W3: block k is [64,4] with column k = W3.
    # Accumulating 4 steps into one [4,BL] psum puts step 4g+k's logit on
    # partition k, so one sigmoid covers 4 steps.
    w_a3 = nc.dram_tensor("w_a3", [64, 16], F32R, kind="ExternalInput")
    w_d1a = nc.dram_tensor("w_d1a", [P, 256], F32R, kind="ExternalInput")
    w_d1b = nc.dram_tensor("w_d1b", [P, 256], F32R, kind="ExternalInput")
    w_d2a = nc.dram_tensor("w_d2a", [P, P], F32R, kind="ExternalInput")
    w_d2b = nc.dram_tensor("w_d2b", [P, P], F32R, kind="ExternalInput")
    w_f = nc.dram_tensor("w_f", [P, 1], F32R, kind="ExternalInput")
    rowsel = nc.dram_tensor("rowsel", [4, 512], F32R, kind="ExternalInput")
    ident = nc.dram_tensor("ident", [P, P], F32R, kind="ExternalInput")
    ones_r = nc.dram_tensor("ones_r", [1, BL], F32R, kind="ExternalInput")
    brow = nc.dram_tensor("brow", [1, 512], F32R, kind="ExternalInput")
    biases = nc.dram_tensor("biases", [P, 16], F32, kind="ExternalInput")
    y_out = nc.dram_tensor("y", [1, BL], F32, kind="ExternalOutput")
    if debug:
        hg_out = nc.dram_tensor("hg", [P, BL], F32R, kind="ExternalOutput")
        h2_out = nc.dram_tensor("h2f", [P, BL], F32R, kind="ExternalOutput")

    with tile.TileContext(nc) as tc:
        import contextlib

        ctx = contextlib.ExitStack()
        with ctx:
            wp = ctx.enter_context(tc.tile_pool(name="wp", bufs=1))
            xp = ctx.enter_context(tc.tile_pool(name="xp", bufs=4))
            hsp = ctx.enter_context(tc.tile_pool(name="hsp", bufs=8))
            gp = ctx.enter_context(tc.tile_pool(name="gp", bufs=2))
            ps = ctx.enter_context(tc.tile_pool(name="ps", bufs=1, space="PSUM"))

            # ---------------- load weights/biases
            def wtile(name, dram, shape, dt=F32R):
                t = wp.tile(shape, dt, name=name, tag=name)
                nc.sync.dma_start(t[:], dram[:])
                return t

            gw = wtile("gw", w_gw, [P, 3 * U])
            gu = wtile("gu", w_gu, [P, 3 * U])
            aw = wtile("aw", w_aw, [P, 3 * U])
            au = wtile("au", w_au, [P, 3 * U])
            ax = wtile("ax", w_ax, [P, P])
            am = wtile("am", w_am, [P, P])
            abc = wtile("abc", w_abc, [P, P])
            a2w = wtile("a2w", w_a2, [P, 64])
            a3w = wtile("a3w", w_a3, [64, 16])
            d1a = wtile("d1a", w_d1a, [P, 256])
            d1b = wtile("d1b", w_d1b, [P, 256])
            d2a = wtile("d2a", w_d2a, [P, P])
            d2b = wtile("d2b", w_d2b, [P, P])
            fw = wtile("fw", w_f, [P, 1])
            rsel = wtile("rsel", rowsel, [4, 512])
            idt = wtile("idt", ident, [P, P])
            ones = wtile("ones", ones_r, [1, BL])
            brw = wtile("brw", brow, [1, 512])
            bia = wtile("bia", biases, [P, 16], F32)
            news_r = wtile("news_r", news, [P, BL])

            def bap(col, rows=P):
                return bia[0:rows, col : col + 1]

            # ---------------- state
            hs_tiles = {}   # t -> tile [P, BL] f32r (GRU outputs, ring of 8)
            h2_tiles = {}   # s -> tile (AUGRU state, ring of 2)
            ats_tiles = {}  # group g -> [4, BL] tile
            a3_psums = {}   # group g -> [4, BL] psum

            zero_h = hsp.tile([P, BL], F32R, name="h_init", tag="hs")
            nc.vector.memset(zero_h[:].bitcast(F32), 0.0)
            zero_h2 = gp.tile([P, BL], F32R, name="h2_init", tag="h2")
            nc.vector.memset(zero_h2[:].bitcast(F32), 0.0)
            hs_tiles[-1] = zero_h
            h2_tiles[-1] = zero_h2

            mm = nc.tensor.matmul

            for i in range(NITER):
                t = i if i < T else None
                j = i - SKEW_ATT if 0 <= i - SKEW_ATT < T else None
                s = i - SKEW_AU if 0 <= i - SKEW_AU < T else None

                # ---------------- GRU step t
                if t is not None:
                    h_prev = hs_tiles[t - 1]
                    x_t = xp.tile([P, BL], F32R, name=f"x{t}", tag="x")
                    nc.sync.dma_start(x_t[:], hist[t])

                    if CFG["merge_zr"]:
                        pp = ps.tile([P, 2 * BL], F32, name=f"pzr{t}", tag="g12")
                        for half, w0 in enumerate([0, U]):
                            sl = pp[:, half * BL : (half + 1) * BL]
                            mm(sl, brw[0:1, half * P : (half + 1) * P], ones[:],
                               start=True, stop=False)
                            mm(sl, gw[:, w0 : w0 + U], x_t[:], start=False, stop=False)
                            mm(sl, gu[:, w0 : w0 + U], h_prev[:], start=False, stop=True)
                        gzr = gp.tile([P, 2 * BL], GD, name=f"gzr{t}", tag="gzr")
                        nc.scalar.activation(gzr[:], pp[:], AF.Sigmoid)
                        z, r = gzr[:, 0:BL], gzr[:, BL : 2 * BL]
                    else:
                        pz = ps.tile([P, BL], F32, name=f"pz{t}", tag="g12")
                        mm(pz[:], gw[:, 0:U], x_t[:], start=True, stop=False)
                        mm(pz[:], gu[:, 0:U], h_prev[:], start=False, stop=True)
                        pr = ps.tile([P, BL], F32, name=f"pr{t}", tag="g12b")
                        mm(pr[:], gw[:, U : 2 * U], x_t[:], start=True, stop=False)
                        mm(pr[:], gu[:, U : 2 * U], h_prev[:], start=False, stop=True)
                        zt = gp.tile([P, BL], GD, name=f"z{t}", tag="gzr")
                        nc.scalar.activation(zt[:], pz[:], AF.Sigmoid, bias=bap(BZ))
                        rt = gp.tile([P, BL], GD, name=f"r{t}", tag="gzrb")
                        nc.scalar.activation(rt[:], pr[:], AF.Sigmoid, bias=bap(BR))
                        z, r = zt[:], rt[:]

                    pxh = ps.tile([P, BL], F32, name=f"pxh{t}", tag="g3")
                    mm(pxh[:], gw[:, 2 * U : 3 * U], x_t[:], start=True, stop=False)
                    phh = ps.tile([P, BL], F32, name=f"phh{t}", tag="g4")
                    mm(phh[:], gu[:, 2 * U : 3 * U], h_prev[:], start=True, stop=True)

                    t1 = gp.tile([P, BL], F32R, name=f"t1{t}", tag="t1")
                    _eng(nc, "t1").scalar_tensor_tensor(
                        t1[:], phh[:], bap(B1H), r, OP.add, OP.mult
                    )
                    # fold r*(hh+b1h) into the candidate PSUM; tanh reads PSUM
                    mm(pxh[:], idt[:], t1[:], start=False, stop=True)
                    hc = gp.tile([P, BL], CD, name=f"hc{t}", tag="hc")
                    nc.scalar.activation(hc[:], pxh[:], AF.Tanh, bias=bap(B0H))

                    p = gp.tile([P, BL], CD, name=f"p{t}", tag="p")
                    _eng(nc, "p").tensor_mul(p[:], z, h_prev[:])
                    # qn = (z-1)*hc = -(1-z)*hc, folded in one op
                    qn = gp.tile([P, BL], CD, name=f"qn{t}", tag="q")
                    _eng(nc, "q").scalar_tensor_tensor(
                        qn[:], z, 1.0, hc[:], OP.subtract, OP.mult
                    )
                    hn = hsp.tile([P, BL], F32R, name=f"h{t}", tag="hs")
                    _eng(nc, "hn").tensor_sub(hn[:], p[:], qn[:])
                    hs_tiles[t] = hn
                    if t >= 8:
                        del hs_tiles[t - 8]

                # ---------------- AUGRU step s
                if s is not None:
                    h2_prev = h2_tiles[s - 1]
                    hs_s = hs_tiles[s]
                    if CFG["merge_ur"]:
                        pp2 = ps.tile([P, 2 * BL], F32, name=f"pur{s}", tag="g12")
                        for half, w0 in enumerate([0, U]):
                            sl = pp2[:, half * BL : (half + 1) * BL]
                            mm(sl, brw[0:1, 256 + half * P : 256 + (half + 1) * P],
                               ones[:], start=True, stop=False)
                            mm(sl, aw[:, w0 : w0 + U], hs_s[:], start=False, stop=False)
                            mm(sl, au[:, w0 : w0 + U], h2_prev[:], start=False, stop=True)
                        gur = gp.tile([P, 2 * BL], GD, name=f"gur{s}", tag="gur")
                        nc.scalar.activation(gur[:], pp2[:], AF.Sigmoid)
                        u, r2 = gur[:, 0:BL], gur[:, BL : 2 * BL]
                    else:
                        pu = ps.tile([P, BL], F32, name=f"pu{s}", tag="g12")
                        mm(pu[:], aw[:, 0:U], hs_s[:], start=True, stop=False)
                        mm(pu[:], au[:, 0:U], h2_prev[:], start=False, stop=True)
                        pr2 = ps.tile([P, BL], F32, name=f"pr2{s}", tag="g12b")
                        mm(pr2[:], aw[:, U : 2 * U], hs_s[:], start=True, stop=False)
                        mm(pr2[:], au[:, U : 2 * U], h2_prev[:], start=False, stop=True)
                        ut = gp.tile([P, BL], GD, name=f"u{s}", tag="gur")
                        nc.scalar.activation(ut[:], pu[:], AF.Sigmoid, bias=bap(BU))
                        r2t = gp.tile([P, BL], GD, name=f"r2{s}", tag="gurb")
                        nc.scalar.activation(r2t[:], pr2[:], AF.Sigmoid, bias=bap(BR2))
                        u, r2 = ut[:], r2t[:]

                    pxc = ps.tile([P, BL], F32, name=f"pxc{s}", tag="g3")
                    mm(pxc[:], aw[:, 2 * U : 3 * U], hs_s[:], start=True, stop=False)
                    prc = ps.tile([P, BL], F32, name=f"prc{s}", tag="g4")
                    mm(prc[:], au[:, 2 * U : 3 * U], h2_prev[:], start=True, stop=True)

                    t2 = gp.tile([P, BL], F32R, name=f"t2{s}", tag="t2")
                    _eng(nc, "t2").tensor_mul(t2[:], prc[:], r2)
                    mm(pxc[:], idt[:], t2[:], start=False, stop=True)
                    c = gp.tile([P, BL], CD, name=f"c{s}", tag="c")
                    nc.scalar.activation(c[:], pxc[:], AF.Tanh, bias=bap(BC))

                    g = s // 4
                    pat = ps.tile([P, BL], F32, name=f"pat{s}", tag="a2at")
                    mm(pat[:], rsel[:, P * (s % 4) : P * (s % 4) + P],
                       ats_tiles[g][:], start=True, stop=True)
                    u_ = gp.tile([P, BL], GD, name=f"u_{s}", tag="u_")
                    _eng(nc, "u_").tensor_mul(u_[:], u, pat[:])
                    # p2n = (u_-1)*h2 = -(1-u_)*h2, folded in one op
                    p2n = gp.tile([P, BL], CD, name=f"p2n{s}", tag="p2")
                    _eng(nc, "p2").scalar_tensor_tensor(
                        p2n[:], u_[:], 1.0, h2_prev[:], OP.subtract, OP.mult
                    )
                    q2 = gp.tile([P, BL], CD, name=f"q2{s}", tag="q2")
                    _eng(nc, "q2").tensor_mul(q2[:], u_[:], c[:])
                    hn2 = gp.tile([P, BL], F32R, name=f"h2_{s}", tag="h2")
                    _eng(nc, "hn2").tensor_sub(hn2[:], q2[:], p2n[:])
                    h2_tiles[s] = hn2
                    if s - 2 in h2_tiles:
                        del h2_tiles[s - 2]

                # ---------------- attention step j
                if j is not None:
                    hs_j = hs_tiles[j]
                    mmj = gp.tile([P, BL], F32R, name=f"mmj{j}", tag="mmj")
                    _eng(nc, "mmj").tensor_mul(mmj[:], hs_j[:], news_r[:])
                    pa1 = ps.tile([P, BL], F32, name=f"pa1{j}", tag="a1")
                    mm(pa1[:], ax[:], hs_j[:], start=True, stop=False)
                    mm(pa1[:], am[:], mmj[:], start=False, stop=False)
                    mm(pa1[:], abc[:], news_r[:], start=False, stop=True)
                    a1 = gp.tile([P, BL], F32R, name=f"a1{j}", tag="a1s")
                    if CFG["relu1_act"]:
                        nc.scalar.activation(a1[:], pa1[:], AF.Relu, bias=bap(B1))
                    else:
                        nc.vector.tensor_scalar(
                            a1[:], pa1[:], bap(B1), 0.0, OP.add, OP.max
                        )
                    pa2 = ps.tile([64, BL], F32, name=f"pa2{j}", tag="a2at")
                    mm(pa2[:], a2w[:], a1[:], start=True, stop=True)
                    a2 = gp.tile([64, BL], F32R, name=f"a2{j}", tag="a2s")
                    _eng(nc, "relu2").tensor_scalar(
                        a2[:], pa2[:], bap(B2, rows=64), 0.0, OP.add, OP.max
                    )
                    g = j // 4
                    k4 = j % 4
                    if k4 == 0:
                        a3_psums[g] = ps.tile([4, BL], F32, name=f"pa3{g}", tag="a3")
                    mm(
                        a3_psums[g][:],
                        a3w[:, 4 * k4 : 4 * k4 + 4],
                        a2[:],
                        start=(k4 == 0),
                        stop=(k4 == 3 or j == T - 1),
                    )
                    if k4 == 3 or j == T - 1:
                        k = k4 + 1
                        ats = gp.tile([4, BL], F32R, name=f"ats{g}", tag="ats")
                        nc.scalar.activation(
                            ats[0:k, :],
                            a3_psums[g][0:k, :],
                            AF.Sigmoid,
                            bias=bap(B3, rows=k),
                        )
                        ats_tiles[g] = ats

            # ---------------- deep head
            h2f = h2_tiles[T - 1]
            if debug:
                nc.sync.dma_start(hg_out[:], hs_tiles[T - 1][:])
                nc.sync.dma_start(h2_out[:], h2f[:])

            o1 = gp.tile([P, 2 * BL], F32R, name="o1", tag="o1")
            for mch in range(2):
                po = ps.tile([P, BL], F32, name=f"po1_{mch}", tag="g3")
                mm(po[:], d1a[:, mch * P : (mch + 1) * P], h2f[:], start=True, stop=False)
                mm(po[:], d1b[:, mch * P : (mch + 1) * P], news_r[:], start=False, stop=True)
                nc.scalar.activation(
                    o1[:, mch * BL : (mch + 1) * BL],
                    po[:],
                    AF.Lrelu,
                    bias=bap(DB1A + mch),
                    alpha=LEAKY,
                )
            po2 = ps.tile([P, BL], F32, name="po2", tag="g4")
            mm(po2[:], d2a[:], o1[:, 0:BL], start=True, stop=False)
            mm(po2[:], d2b[:], o1[:, BL : 2 * BL], start=False, stop=True)
            o2 = gp.tile([P, BL], F32R, name="o2", tag="o2")
            nc.scalar.activation(o2[:], po2[:], AF.Lrelu, bias=bap(DB2), alpha=LEAKY)
            py = ps.tile([1, BL], F32, name="py", tag="a3")
            mm(py[:], fw[:], o2[:], start=True, stop=True)
            y_sb = gp.tile([1, BL], F32, name="y_sb", tag="ysb")
            nc.scalar.activation(y_sb[:], py[:], AF.Sigmoid, bias=bap(FB, rows=1))
            nc.sync.dma_start(y_out[:], y_sb[:])

    nc.compile()
    return nc


def _a3_onehot(att_W3):
    w = np.zeros((64, 16), np.float32)
    for k in range(4):
        w[:, 4 * k + k] = att_W3[:, 0]
    return w


def prep_inputs(inputs_np, gru_W, gru_U, gru_b, att_W1, att_b1, att_W2, att_b2,
                att_W3, att_b3, au_Wu, au_bu, au_Uu, au_Wr, au_br, au_Ur,
                au_Wc, au_bc, au_Uc, bn_gamma, bn_beta, bn_mean, bn_var,
                d_W1, d_b1, d_W2, d_b2, f_W, f_b):
    """Host-side preprocessing. Returns per-core input maps."""
    f32 = np.float32

    biases = np.zeros((P, 16), f32)
    biases[:, BZ] = gru_b[0, 0:U] + gru_b[1, 0:U]
    biases[:, BR] = gru_b[0, U : 2 * U] + gru_b[1, U : 2 * U]
    biases[:, BU] = au_bu
    biases[:, BR2] = au_br
    biases[:, B1H] = gru_b[1, 2 * U : 3 * U]
    biases[:, B0H] = gru_b[0, 2 * U : 3 * U]
    biases[:, BC] = au_bc
    biases[:, B1] = att_b1
    biases[0:64, B2] = att_b2
    biases[0:4, B3] = att_b3[0]

    # gate-pair bias rows for the K=1 bias matmuls: [bz | br | bu | br2]
    brow = np.zeros((1, 512), f32)
    brow[0, 0:P] = biases[:, BZ]
    brow[0, P : 2 * P] = biases[:, BR]
    brow[0, 2 * P : 3 * P] = biases[:, BU]
    brow[0, 3 * P : 4 * P] = biases[:, BR2]

    # BN folded into layer 1
    s = (bn_gamma / np.sqrt(bn_var + 1e-3)).astype(f32)
    tt = (bn_beta - bn_mean * s).astype(f32)
    W1p = (s[:, None] * d_W1).astype(f32)
    b1p = (tt @ d_W1 + d_b1).astype(f32)
    biases[:, DB1A] = b1p[0:P]
    biases[:, DB1B] = b1p[P : 2 * P]
    biases[:, DB2] = d_b2
    biases[0, FB] = f_b[0]

    rowsel = np.zeros((4, 512), f32)
    for k in range(4):
        rowsel[k, P * k : P * k + P] = 1.0

    shared = {
        "w_gw": np.ascontiguousarray(gru_W, f32),
        "w_gu": np.ascontiguousarray(gru_U, f32),
        "w_aw": np.ascontiguousarray(np.concatenate([au_Wu, au_Wr, au_Wc], axis=1), f32),
        "w_au": np.ascontiguousarray(np.concatenate([au_Uu, au_Ur, au_Uc], axis=1), f32),
        "w_ax": np.ascontiguousarray(att_W1[0:P] - att_W1[P : 2 * P], f32),
        "w_am": np.ascontiguousarray(att_W1[3 * P : 4 * P], f32),
        "w_abc": np.ascontiguousarray(att_W1[P : 2 * P] + att_W1[2 * P : 3 * P], f32),
        "w_a2": np.ascontiguousarray(att_W2, f32),
        "w_a3": _a3_onehot(att_W3),
        "w_d1a": np.ascontiguousarray(W1p[0:P], f32),
        "w_d1b": np.ascontiguousarray(W1p[P : 2 * P], f32),
        "w_d2a": np.ascontiguousarray(d_W2[0:P], f32),
        "w_d2b": np.ascontiguousarray(d_W2[P : 2 * P], f32),
        "w_f": np.ascontiguousarray(f_W, f32),
        "rowsel": rowsel,
        "ident": np.eye(P, dtype=f32),
        "ones_r": np.ones((1, BL), f32),
        "brow": brow,
        "biases": biases,
    }

    in_maps = []
    for c in range(NCORES):
        sh = inputs_np[c * BL : (c + 1) * BL]  # [BL, T+1, D]
        hist_t = np.ascontiguousarray(sh[:, :T, :].transpose(1, 2, 0), f32)  # [T,D,BL]
        news_t = np.ascontiguousarray(sh[:, T, :].T, f32)  # [D, BL]
        m = dict(shared)
        m["hist"] = hist_t
        m["news"] = news_t
        in_maps.append(m)
    return in_maps


_NC_CACHE = {}


def get_nc(debug=False):
    key = (debug,)
    if key not in _NC_CACHE:
        _NC_CACHE[key] = build_nc(debug=debug)
    return _NC_CACHE[key]


def kernel(**inputs):
    inputs = {k: np.asarray(v) for k, v in inputs.items()}
    in_maps = prep_inputs(
        inputs["inputs"], inputs["gru_W"], inputs["gru_U"], inputs["gru_b"],
        inputs["att_W1"], inputs["att_b1"], inputs["att_W2"], inputs["att_b2"],
        inputs["att_W3"], inputs["att_b3"], inputs["au_Wu"], inputs["au_bu"],
        inputs["au_Uu"], inputs["au_Wr"], inputs["au_br"], inputs["au_Ur"],
        inputs["au_Wc"], inputs["au_bc"], inputs["au_Uc"], inputs["bn_gamma"],
        inputs["bn_beta"], inputs["bn_mean"], inputs["bn_var"], inputs["d_W1"],
        inputs["d_b1"], inputs["d_W2"], inputs["d_b2"], inputs["f_W"],
        inputs["f_b"],
    )
    nc = get_nc(debug=CFG["debug"])
    res = run_bass_kernel_spmd(nc, in_maps, list(range(NCORES)))
    y = np.concatenate(
        [res.results[c]["y"].reshape(-1)[:, None] for c in range(NCORES)], axis=0
    ).astype(np.float32)
    return y
